# revision 12
# baseline (speedup 1.0000x reference)
"""Trainium2 Bass kernel for nn_MultiHeadAttention (B=2, S=2048, D=1024, H=16).

Reference semantics (note the *raw-view* head split):
    q = query @ Wq.T + bq                  # [B, S, D]
    q = q.reshape(B, H, S, DK)             # raw view: head h = rows [h*128,(h+1)*128) of q[b]
                                           #   viewed as [2048, 64]
    scores = q @ k.T / sqrt(DK), causal mask, softmax
    ctx    = softmax @ v                   # [B, H, S, DK]
    out    = ctx.transpose(0,2,1,3).reshape(B,S,D) @ Wo.T + bo

Sharding: 8 cores = 2 batches x 4 head-groups.  Core (b, g) owns heads
[4g, 4g+4) of batch b, i.e. rows [512g, 512g+512) of the QKV projections.
Each core computes its 4 heads' attention plus its partial contribution
C_heads @ Wo[:, head cols].T of the output projection; the host sums the 4
partials per batch and adds bo.

v3 dataflow (bf16 compute, fp32 psum accumulate):
  - x and W stream in bf16 with p-major DRAM layouts so each projection
    needs only a few large DMAs (HWDGE descriptor-gen is ~630ns/DMA).
  - q,k,v projections all emit feature-major psum tiles [f,r]; a strided
    scatter builds per-head-pair bf16 tensors [128=2 heads x 64 dk, 2048].
    q/k use raw position order; v uses a transpose-friendly order so that
    one XBAR dma_start_transpose per head yields vtmp[h] = [128 kpos,
    16 ktile x 64 dk] (v tile j = vtmp[:, 64j:64j+64], contiguous).
  - scores_T = kT.T @ qT per (k-tile 128, q-window 512); the two heads of
    a pair run as row-tiled (K=64) matmuls in disjoint PE quadrants.  On
    the causal diagonal only cols >= 128*d are computed (partial-N).
    exp on ACT writes bf16 pt; the boundary block gets a tri01 multiply
    on DVE.  Scores+exp run LOOKAHEAD steps ahead of ctx (the first
    LOOKAHEAD are interleaved into the v projection) so ACT is never the
    per-step critical path.
  - ctx_T[dk, q] accumulates with M=64 matmuls; the softmax denominator
    row accumulates via a concurrent col-tiled M=1 ones matmul into
    psum row 64 (tile_position (0,64), start=False rides the ctx clear).
  - normalize with reciprocal + gpsimd partition_broadcast, then the
    output projection out[s, o] = sum_pairs ctxT_pair.T @ WoT_pair (fp16).
"""

import os
import sys

import numpy as np

_TRN_REPO = "/opt/trn_rl_repo"
if _TRN_REPO not in sys.path:
    sys.path.insert(0, _TRN_REPO)

B, S, D, H = 2, 2048, 1024, 16
DK = D // H  # 64
N_CORES = 8
HEADS_PER_CORE = H // 4  # 4
ROWS_PER_CORE = HEADS_PER_CORE * (S // H)  # 512 rows of the projection output
QW = 512  # q-position window (psum free-dim)
KT = 128  # k-position tile
LOOKAHEAD = 8  # scores/exp pipeline depth (steps)


def _build_program(repeat=1, phases=3, ivl=True):
    import concourse.bass as bass
    import concourse.bacc as bacc
    import concourse.mybir as mybir
    from concourse.tile import TileContext
    from concourse import library_config

    f32 = mybir.dt.float32
    bf16 = mybir.dt.bfloat16
    f16 = mybir.dt.float16
    Exp = mybir.ActivationFunctionType.Exp
    Copy = mybir.ActivationFunctionType.Copy
    Identity = mybir.ActivationFunctionType.Identity
    MUL = mybir.AluOpType.mult
    ADD = mybir.AluOpType.add

    nc = bacc.Bacc("TRN2", target_bir_lowering=False, debug=False)

    # ---- DRAM parameters (host pre-tiled / pre-transposed) ----
    # x: [128, 8 ktile, 512 rows] p-major; w: [2 fhalf, 128, 8 ktile, 512 f]
    xq = nc.dram_tensor("xq", [128, 8, QW], bf16, kind="ExternalInput")
    xk = nc.dram_tensor("xk", [128, 8, QW], bf16, kind="ExternalInput")
    xv = nc.dram_tensor("xv", [128, 8, QW], bf16, kind="ExternalInput")
    wq = nc.dram_tensor("wq", [2, 128, 8, 512], bf16, kind="ExternalInput")
    wk = nc.dram_tensor("wk", [2, 128, 8, 512], bf16, kind="ExternalInput")
    wv = nc.dram_tensor("wv", [2, 128, 8, 512], bf16, kind="ExternalInput")
    wo = nc.dram_tensor("wo", [2, 128, 1024], bf16, kind="ExternalInput")
    bqd = nc.dram_tensor("bqd", [128, 16], f32, kind="ExternalInput")
    bkd = nc.dram_tensor("bkd", [128, 16], f32, kind="ExternalInput")
    bvd = nc.dram_tensor("bvd", [128, 16], f32, kind="ExternalInput")
    tri = nc.dram_tensor("tri", [128, 128], bf16, kind="ExternalInput")
    out = nc.dram_tensor("out", [S, D], f16, kind="ExternalOutput")

    with TileContext(nc) as tc:
      from contextlib import ExitStack
      with ExitStack() as stack:
        persist = stack.enter_context(tc.tile_pool(name="persist", bufs=1))
        small = stack.enter_context(tc.tile_pool(name="small", bufs=4))
        xp = stack.enter_context(tc.tile_pool(name="xp", bufs=2))
        wp = stack.enter_context(tc.tile_pool(name="wp", bufs=2))
        ptp = stack.enter_context(tc.tile_pool(name="ptp", bufs=12))
        wop = stack.enter_context(tc.tile_pool(name="wop", bufs=1))
        osb = stack.enter_context(tc.tile_pool(name="osb", bufs=3))
        for rep in range(repeat):
            # persistent tiles
            qpair = [persist.tile([128, S], bf16, tag=f"qpair{p}", name=f"qpair{p}") for p in range(2)]
            kpair = [persist.tile([128, S], bf16, tag=f"kpair{p}", name=f"kpair{p}") for p in range(2)]
            vT = [persist.tile([128, S], bf16, tag=f"vT{p}", name=f"vT{p}") for p in range(2)]
            ctxT = [persist.tile([128, S], bf16, tag=f"ctxT{p}", name=f"ctxT{p}") for p in range(2)]
            # vtmp[h]: [128 kpos, 16 ktile x 64 dk] (ctx lhsT source)
            vtmp = [persist.tile([128, 1024], bf16, tag=f"vtmp{h}", name=f"vtmp{h}") for h in range(4)]
            # vh65[h]: per ktile j cols [65j, 65j+64) = v data, col 65j+64 = ones
            vh65 = [persist.tile([128, 16 * 65], bf16, tag=f"vh65_{h}", name=f"vh65_{h}") for h in range(4)]
            tri01 = persist.tile([128, 128], bf16, tag="tri01")
            bq_t = persist.tile([128, 16], f32, tag="bq_t")
            bk_t = persist.tile([128, 16], f32, tag="bk_t")
            bv_t = persist.tile([128, 16], f32, tag="bv_t")
            nc.sync.dma_start(out=tri01[:], in_=tri[:])
            nc.sync.dma_start(out=bq_t[:], in_=bqd[:])
            nc.sync.dma_start(out=bk_t[:], in_=bkd[:])
            nc.sync.dma_start(out=bv_t[:], in_=bvd[:])

            wo_t = []
            for pair in range(2):
                t = wop.tile([128, 1024], bf16, tag=f"wo{pair}", name=f"wo{pair}")
                nc.sync.dma_start(out=t[:], in_=wo[pair])
                wo_t.append(t)

            # ---------------- attention step plumbing ----------------
            steps = []
            for pair in range(2):
                for qi in range(4):
                    nkt = 4 * qi + 4
                    for kj in range(nkt):
                        steps.append((qi, pair, kj, nkt))

            sp_map = {}

            def scores(pair, qi, kj, scps):
                # one [128, 1024] psum duo = both heads' scores for kj;
                # on the causal diagonal only cols >= 128*d are computed
                d = kj - 4 * qi
                off = 128 * d if d > 0 else 0
                sp = scps.tile([128, 2 * QW], f32, tag="sduo")
                for h2 in range(2):
                    nc.tensor.matmul(
                        sp[:, h2 * QW + off : (h2 + 1) * QW],
                        kpair[pair][h2 * 64 : h2 * 64 + 64, kj * KT : (kj + 1) * KT],
                        qpair[pair][h2 * 64 : h2 * 64 + 64, qi * QW + off : (qi + 1) * QW],
                        start=True,
                        stop=True,
                    )
                return sp

            def emit_scores_exp(si, scps):
                qi, pair, kj, nkt = steps[si]
                sp = scores(pair, qi, kj, scps)
                d = kj - 4 * qi
                off = 128 * d if d > 0 else 0
                pt = ptp.tile([128, 2 * QW], bf16, tag="ptduo")
                s3 = sp[:].rearrange("p (h x) -> p h x", h=2)
                p3 = pt[:].rearrange("p (h x) -> p h x", h=2)
                if d >= 0:
                    nc.scalar.activation(p3[:, :, off:], s3[:, :, off:], Exp)
                else:
                    nc.scalar.activation(pt[:], sp[:], Exp)
                sp_map[si] = pt

            # ---------------- Phase 1: projections ----------------
            with (
                tc.tile_pool(name=f"scps{rep}", bufs=2, space="PSUM") as scps,
            ):
              with (
                  tc.tile_pool(name=f"pps{rep}", bufs=2, space="PSUM") as pps,
              ):
                def qk_projection(xdram, wdram, bias_t, dst_ap_fn, group_hook=None):
                    xall = xp.tile([128, 8 * QW], bf16, tag="xall")
                    for c in range(2):
                        nc.sync.dma_start(
                            out=xall[:, c * 4 * QW : (c + 1) * 4 * QW],
                            in_=xdram[:, c * 4 : (c + 1) * 4, :],
                        )
                    for fh in range(2):
                        wall = wp.tile([128, 8 * 512], bf16, tag="wall")
                        for c in range(2):
                            nc.sync.dma_start(
                                out=wall[:, c * 4 * 512 : (c + 1) * 4 * 512],
                                in_=wdram[fh, :, c * 4 : (c + 1) * 4, :],
                            )
                        for f4 in range(4):
                            f = fh * 4 + f4
                            ps = pps.tile([128, QW], f32, tag="proj")
                            for i in range(8):
                                nc.tensor.matmul(
                                    ps[:],
                                    wall[:, i * 512 + f4 * 128 : i * 512 + (f4 + 1) * 128],
                                    xall[:, i * QW : (i + 1) * QW],
                                    start=(i == 0),
                                    stop=(i == 7),
                                )
                            # scatter: psum [f 128, r 512] -> pair tiles, strided
                            for c2 in range(2):
                                chunk = 2 * f + c2
                                src_half = ps[c2 * 64 : (c2 + 1) * 64, :]
                                bias_ap = bias_t[c2 * 64 : (c2 + 1) * 64, chunk : chunk + 1]
                                for h in range(4):
                                    dst = dst_ap_fn(h, chunk)
                                    if h < 3:
                                        nc.vector.tensor_scalar(
                                            out=dst,
                                            in0=src_half[:, h * 128 : (h + 1) * 128],
                                            scalar1=bias_ap,
                                            scalar2=None,
                                            op0=ADD,
                                        )
                                    else:
                                        nc.scalar.activation(
                                            dst,
                                            src_half[:, h * 128 : (h + 1) * 128],
                                            Identity,
                                            bias=bias_ap,
                                        )
                            if group_hook is not None:
                                group_hook(f)

                def qk_dst(dest_pair):
                    def fn(h, chunk):
                        return (
                            dest_pair[h // 2][(h % 2) * 64 : (h % 2) * 64 + 64, :]
                            .rearrange("p (r c) -> p r c", c=16)[:, :, chunk]
                        )
                    return fn

                qk_projection(xq, wq, bq_t, qk_dst(qpair))
                qk_projection(xk, wk, bk_t, qk_dst(kpair))
                # v projection (raw position order, same scatter as q/k):
                # interleave the first LOOKAHEAD scores+exp
                qk_projection(
                    xv, wv, bv_t, qk_dst(vT),
                    group_hook=lambda f: emit_scores_exp(f, scps),
                )

                # vtmp[h] via one XBAR dma transpose per head, then a
                # strided DVE copy into vh65 (+ softmax-denominator ones col)
                for h in range(4):
                    nc.sync.dma_start(
                        out=vtmp[h][:].rearrange("p (e d) -> p e d", d=64),
                        in_=vT[h // 2][(h % 2) * 64 : (h % 2) * 64 + 64, :],
                        transpose=True,
                    )
                for h in range(4):
                    nc.vector.tensor_copy(
                        out=vh65[h][:].rearrange("p (j e) -> p j e", e=65)[:, :, 0:64],
                        in_=vtmp[h][:].rearrange("p (e d) -> p e d", d=64),
                    )
                    nc.vector.tensor_scalar(
                        out=vh65[h][:].rearrange("p (j e) -> p j e", e=65)[:, :, 64],
                        in0=vh65[h][:, 0:16],
                        scalar1=0.0,
                        scalar2=1.0,
                        op0=MUL,
                        op1=ADD,
                    )

              # ---------------- Phase 2: attention ----------------
              attn_stack = ExitStack()
              ctxps = attn_stack.enter_context(
                  tc.tile_pool(name=f"ctxps{rep}", bufs=2, space="PSUM")
              )
              vpps = attn_stack.enter_context(
                  tc.tile_pool(name=f"vpps{rep}", bufs=1, space="PSUM")
              )
              emitted_st = set()

              def emit_out_stile(st, on_act=False):
                    emitted_st.add(st)
                    ostage = osb.tile([128, 1024], f16, tag="ostage", name="ostage")
                    for og in range(2):
                        ps = vpps.tile([128, 512], f32, tag="vproj", name="vproj")
                        for pair in range(2):
                            nc.tensor.matmul(
                                ps[:],
                                ctxT[pair][:, st * 128 : (st + 1) * 128],
                                wo_t[pair][:, og * 512 : (og + 1) * 512],
                                start=(pair == 0),
                                stop=(pair == 1),
                            )
                        if on_act:
                            nc.scalar.activation(
                                ostage[:, og * 512 : (og + 1) * 512], ps[:], Copy
                            )
                        else:
                            nc.vector.tensor_copy(
                                out=ostage[:, og * 512 : (og + 1) * 512], in_=ps[:]
                            )
                    (nc.sync if st % 2 == 0 else nc.scalar).dma_start(
                        out=out[st * 128 : (st + 1) * 128, :], in_=ostage[:]
                    )

              fillers = []
              cps_map = {}
              for si, (qi, pair, kj, nkt) in enumerate(steps):
                    if si % 2 == 1 and fillers:
                        fillers.pop(0)()
                    if si + LOOKAHEAD < len(steps):
                        emit_scores_exp(si + LOOKAHEAD, scps)
                    pt = sp_map.pop(si)
                    if kj == 0:
                        cps_map[(qi, pair)] = [
                            ctxps.tile([DK + 1, QW], f32, tag=f"ctx{h2}",
                                       name=f"ctx{h2}", bufs=(2 if h2 == 0 else 1))
                            for h2 in range(2)
                        ]
                    cps = cps_map[(qi, pair)]
                    d = kj - 4 * qi
                    off = 128 * d if d > 0 else 0
                    if d >= 0:
                        for h2 in range(2):
                            nc.vector.tensor_tensor(
                                out=pt[:, h2 * QW + off : h2 * QW + off + 128],
                                in0=pt[:, h2 * QW + off : h2 * QW + off + 128],
                                in1=tri01[:],
                                op=MUL,
                            )
                    for h2 in range(2):
                        h = 2 * pair + h2
                        nc.tensor.matmul(
                            cps[h2][:, off:],
                            vh65[h][:, kj * 65 : kj * 65 + 65],
                            pt[:, h2 * QW + off : (h2 + 1) * QW],
                            start=(kj == 0),
                            stop=(kj == nkt - 1),
                        )
                    if kj == nkt - 1:
                        # normalize: ctxU / denom -> ctxT
                        for h2 in range(2):
                            rec = small.tile([1, QW], f32, tag="rec")
                            nc.vector.reciprocal(rec[:], cps[h2][64:65, :])
                            bc = small.tile([64, QW], f32, tag="bc")
                            nc.gpsimd.partition_broadcast(bc[:], rec[:], channels=64)
                            nc.vector.tensor_tensor(
                                out=ctxT[pair][
                                    h2 * 64 : h2 * 64 + 64, qi * QW : (qi + 1) * QW
                                ],
                                in0=cps[h2][0:64, :],
                                in1=bc[:],
                                op=MUL,
                            )
                        del cps_map[(qi, pair)]
                        if pair == 1 and qi < 3:
                            for st in range(qi * 4, qi * 4 + 4):
                                fillers.append(
                                    lambda st=st: emit_out_stile(st, on_act=False)
                                )
              # flush any fillers that did not get a pop slot
              while fillers:
                    fillers.pop(0)()
              attn_stack.close()

            # ---------------- Phase 3: output projection ----------------
            with (
                tc.tile_pool(name=f"ops{rep}", bufs=3, space="PSUM") as ops,
            ):
                for st in range(16):
                    if st in emitted_st:
                        continue
                    ostage = osb.tile([128, 1024], f16, tag="ostage")
                    for og in range(2):
                        ps = ops.tile([128, 512], f32, tag="ops")
                        for pair in range(2):
                            nc.tensor.matmul(
                                ps[:],
                                ctxT[pair][:, st * 128 : (st + 1) * 128],
                                wo_t[pair][:, og * 512 : (og + 1) * 512],
                                start=(pair == 0),
                                stop=(pair == 1),
                            )
                        nc.scalar.activation(
                            ostage[:, og * 512 : (og + 1) * 512], ps[:], Copy
                        )
                    (nc.sync if st % 2 == 0 else nc.scalar).dma_start(
                        out=out[st * 128 : (st + 1) * 128, :], in_=ostage[:]
                    )

    nc.finalize()
    return nc


_NC_CACHE = {}


def _get_program(repeat=1, phases=3, ivl=True):
    key = (repeat, phases, ivl)
    if key not in _NC_CACHE:
        _NC_CACHE[key] = _build_program(repeat, phases, ivl)
    return _NC_CACHE[key]


def _host_inputs(query, key, value, Wq, bq, Wk, bk, Wv, bv, Wo):
    """Build the 8 per-core input maps (numpy, host-side shard/transpose)."""
    import ml_dtypes

    bf16 = ml_dtypes.bfloat16
    query = np.asarray(query, dtype=np.float32)
    key = np.asarray(key, dtype=np.float32)
    value = np.asarray(value, dtype=np.float32)
    Wq = np.asarray(Wq, dtype=np.float32)
    Wk = np.asarray(Wk, dtype=np.float32)
    Wv = np.asarray(Wv, dtype=np.float32)
    Wo = np.asarray(Wo, dtype=np.float32)
    bq = np.asarray(bq, dtype=np.float32)
    bk = np.asarray(bk, dtype=np.float32)
    bv = np.asarray(bv, dtype=np.float32)

    scale = 1.0 / np.sqrt(np.float32(DK))

    def wtile(WT):  # [1024 i,1024 f] -> [2 fh, 128 ipart, 8 itile, 512 f]
        return np.ascontiguousarray(
            WT.reshape(8, 128, 2, 512).transpose(2, 1, 0, 3)
        ).astype(bf16)

    wq4 = wtile(Wq.T * scale)
    wk4 = wtile(Wk.T)
    wv4 = wtile(Wv.T)
    WoT = np.ascontiguousarray(Wo.T)  # [i, o]

    def dup_bias(b):  # [1024] -> [128, 16] dup layout
        m = b.reshape(16, 64).T  # [64, 16]
        return np.ascontiguousarray(np.vstack([m, m]))

    bqd = dup_bias(bq * scale)
    bkd = dup_bias(bk)
    bvd = dup_bias(bv)
    tri01 = np.ascontiguousarray(np.triu(np.ones((128, 128), np.float32))).astype(bf16)

    def xtile(x):  # [512 rows, 1024 feat] -> [128 ipart, 8 itile, 512 rows]
        return np.ascontiguousarray(
            x.T.reshape(8, 128, QW).transpose(1, 0, 2)
        ).astype(bf16)

    in_maps = []
    for core in range(N_CORES):
        b, g = divmod(core, 4)
        sl = slice(g * ROWS_PER_CORE, (g + 1) * ROWS_PER_CORE)
        wo4 = np.ascontiguousarray(
            WoT[g * 256 : (g + 1) * 256, :]
        ).astype(bf16).reshape(2, 128, 1024)
        in_maps.append(
            {
                "xq": xtile(query[b, sl, :]),
                "xk": xtile(key[b, sl, :]),
                "xv": xtile(value[b, sl, :]),
                "wq": wq4,
                "wk": wk4,
                "wv": wv4,
                "wo": wo4,
                "bqd": bqd,
                "bkd": bkd,
                "bvd": bvd,
                "tri": tri01,
            }
        )
    return in_maps


def run_cores(in_maps, trace=False, trace_kwargs=None, repeat=1):
    """Compile + run the SPMD program on cores 0-7, return BassKernelResults."""
    from concourse.bass_utils import run_bass_kernel_spmd

    nc = _get_program(repeat)
    kwargs = {}
    if trace:
        kwargs["trace"] = True
        if trace_kwargs:
            kwargs["trace_kwargs"] = trace_kwargs
    return run_bass_kernel_spmd(nc, in_maps, core_ids=list(range(N_CORES)), **kwargs)


def kernel(query, key, value, mask, Wq, bq, Wk, bk, Wv, bv, Wo, bo, _trace=False):
    in_maps = _host_inputs(query, key, value, Wq, bq, Wk, bk, Wv, bv, Wo)
    res = run_cores(in_maps, trace=_trace)
    bo = np.asarray(bo, dtype=np.float32)
    out = np.zeros((B, S, D), dtype=np.float32)
    for core in range(N_CORES):
        b = core // 4
        out[b] += res.results[core]["out"].astype(np.float32)
    out += bo[None, None, :]
    kernel.last_results = res
    return out
